# revision 1
# baseline (speedup 1.0000x reference)
"""Trainium2 Bass kernel for nn_MultiHeadAttention (B=2, S=2048, D=1024, H=16, HD=64).

Strategy (8 NeuronCores, tensor-parallel over heads):
  - Each core owns 2 heads (128 of the 1024 q/k/v features).
  - QKV projections computed in transposed layout (features on partitions,
    tokens on the free dim) from a host-pre-transposed bf16 copy of h.
  - Attention computed as S^T = K @ Q^T tiles ([key, query] layout) so the
    softmax denominator and the P@V product both contract over the key
    (partition) axis on the PE.  Softmax has no max-subtraction (scores are
    provably tiny: |s| < 3) and the denominator comes from a ones-column
    appended to V.  Causal masking: block-skipping + a triangular bf16 mask
    multiply on diagonal blocks.
  - Normalisation is deferred until after an AllToAll that converts the
    head-sharded o^T into token-sharded full-feature o^T (each core keeps 512
    tokens).  The unnormalised o^T plus per-(head,token) denominators travel
    through the AllToAll; each core then normalises and applies W_O for its
    512 tokens.  Host concatenates the 8 token shards.
All matmuls in bf16 with fp32 PSUM accumulation.
"""

import numpy as np
import ml_dtypes

import concourse.bass as bass
import concourse.tile as tile
import concourse.mybir as mybir
from concourse import bacc
from concourse.bass_utils import run_bass_kernel_spmd

BF16 = ml_dtypes.bfloat16
F32 = np.float32

B, S, D, H, HD = 2, 2048, 1024, 16, 64
P = 128                      # partitions
TOK = B * S                  # 4096 flattened tokens
DT = D // P                  # 8 d-tiles
NCORES = 8
HLOC = H // NCORES           # 2 heads per core
TOKC = TOK // NCORES         # 512 tokens per core after AllToAll
QC = 1024                    # attention query-chunk width
KT = P                       # key tile = 128
VW = 80                      # per-ktile stride in v_sb ([64 v | 1 ones | 15 pad])

dt_bf = mybir.dt.bfloat16
dt_f32 = mybir.dt.float32
EXP = mybir.ActivationFunctionType.Exp


def _build_nc(single=False, repeat=1):
    # single=True: no collective (replaced by a DRAM->DRAM copy), for
    # single-core timeline simulation / profiling only.
    # repeat>1: run the whole pipeline N times (for wall-clock benchmarking
    # that amortises the host dispatch overhead).
    nc = bacc.Bacc("TRN2", target_bir_lowering=False, debug=False,
                   num_devices=1 if single else NCORES)

    hT = nc.dram_tensor("hT", [P, DT, TOK], dt_bf, kind="ExternalInput")
    woT = nc.dram_tensor("woT", [P, DT, D], dt_bf, kind="ExternalInput")
    wq = nc.dram_tensor("wq", [P, DT, P], dt_bf, kind="ExternalInput")
    wk = nc.dram_tensor("wk", [P, DT, P], dt_bf, kind="ExternalInput")
    wv = nc.dram_tensor("wv", [P, DT, P], dt_bf, kind="ExternalInput")
    bqd = nc.dram_tensor("bq", [P, 1], dt_f32, kind="ExternalInput")
    bkd = nc.dram_tensor("bk", [P, 1], dt_f32, kind="ExternalInput")
    bvd = nc.dram_tensor("bv", [P, 1], dt_f32, kind="ExternalInput")
    bod = nc.dram_tensor("bo", [P, DT], dt_f32, kind="ExternalInput")
    trid = nc.dram_tensor("tri", [P, P], dt_bf, kind="ExternalInput")
    seld = nc.dram_tensor("sel", [H, DT * P], dt_bf, kind="ExternalInput")
    out_t = nc.dram_tensor("out", [D, TOKC], dt_f32, kind="ExternalOutput")

    with tile.TileContext(nc) as tc:
        with (
            tc.tile_pool(name="persist", bufs=1) as persist,
            tc.tile_pool(name="pt_pool", bufs=10) as pt_pool,
            tc.tile_pool(name="outp", bufs=3) as outp,
            tc.tile_pool(name="ps_st", bufs=3, space="PSUM") as ps_st,
            tc.tile_pool(name="ps_ot", bufs=1, space="PSUM") as ps_ot,
            tc.tile_pool(name="dram", bufs=1, space="DRAM") as dram,
        ):
            # ---- resident SBUF tensors -------------------------------------
            hT_sb = persist.tile([P, DT, TOK], dt_bf)
            woT_sb = persist.tile([P, DT, D], dt_bf)
            wq_sb = persist.tile([P, DT, P], dt_bf)
            wk_sb = persist.tile([P, DT, P], dt_bf)
            wv_sb = persist.tile([P, DT, P], dt_bf)
            bq_sb = persist.tile([P, 1], dt_f32)
            bk_sb = persist.tile([P, 1], dt_f32)
            bv_sb = persist.tile([P, 1], dt_f32)
            bo_sb = persist.tile([P, DT], dt_f32)
            tri_sb = persist.tile([P, P], dt_bf)
            sel_sb = persist.tile([H, DT * P], dt_bf)
            qT_sb = persist.tile([P, TOK], dt_bf)
            kT_sb = persist.tile([P, TOK], dt_bf)
            vT_sb = persist.tile([P, TOK], dt_bf)
            v_sb = persist.tile([P, B, HLOC, S // KT, VW], dt_bf)
            # per-head unnormalised o^T (rows 0-63) + denominator (row 64)
            oun0_sb = persist.tile([HD + 1, TOK], dt_bf)
            oun1_sb = persist.tile([HD + 1, TOK], dt_bf)
            oT_sb = persist.tile([P, DT, TOKC], dt_bf)
            den16_sb = persist.tile([H, TOKC], dt_bf)
            rec16_sb = persist.tile([H, TOKC], dt_f32)
            rec16b_sb = persist.tile([H, TOKC], dt_bf)

            # small constants first so the first matmuls are unblocked early
            nc.sync.dma_start(wq_sb[:], wq[:])
            nc.sync.dma_start(wk_sb[:], wk[:])
            nc.sync.dma_start(wv_sb[:], wv[:])
            nc.sync.dma_start(bq_sb[:], bqd[:])
            nc.sync.dma_start(bk_sb[:], bkd[:])
            nc.sync.dma_start(bv_sb[:], bvd[:])
            nc.sync.dma_start(bo_sb[:], bod[:])
            nc.sync.dma_start(tri_sb[:], trid[:])
            nc.sync.dma_start(sel_sb[:], seld[:])
            nc.vector.memset(v_sb[:, :, :, :, HD:HD + 1], 1.0)

            a2a_in0 = dram.tile([NCORES, HD + 1, TOKC], dt_bf)
            a2a_out0 = dram.tile([NCORES, HD + 1, TOKC], dt_bf)
            a2a_in1 = dram.tile([NCORES, HD + 1, TOKC], dt_bf)
            a2a_out1 = dram.tile([NCORES, HD + 1, TOKC], dt_bf)

            PROJ = {
                "q": (wq_sb, bq_sb, qT_sb),
                "k": (wk_sb, bk_sb, kT_sb),
                "v": (wv_sb, bv_sb, vT_sb),
            }

            def proj_one(ch, which):
                # one of q^T/k^T/v^T for tokens [QC*ch, QC*ch+QC)
                w_sb, b_sb, dst = PROJ[which]
                ps = ps_st.tile([P, QC], dt_f32, tag="st", name="ps_proj")
                for a in range(DT):
                    for n in range(2):
                        nc.tensor.matmul(
                            ps[:, 512 * n:512 * n + 512],
                            w_sb[:, a, :],
                            hT_sb[:, a, QC * ch + 512 * n:QC * ch + 512 * n + 512],
                            start=(a == 0),
                            stop=(a == DT - 1),
                        )
                nc.vector.tensor_scalar_add(
                    out=dst[:, QC * ch:QC * ch + QC], in0=ps[:], scalar1=b_sb[:],
                )

            def v_transpose(b2, c):
                # v^T chunk -> [token, feature] tiles (8 k-tiles) + ones col.
                # On the ACT HWDGE queue so it doesn't serialize the SP queue's
                # plain DMAs (xbar mode transitions).
                for hh in range(HLOC):
                    nc.scalar.dma_start_transpose(
                        v_sb[:, b2, hh, 8 * c:8 * c + 8, 0:HD],
                        vT_sb[HD * hh:HD * hh + HD,
                              S * b2 + QC * c:S * b2 + QC * c + QC],
                    )

            def attention(b2, hh, c):
                fb = HD * hh  # feature base of this head in qT/kT
                qbase = S * b2 + QC * c
                nkt = (QC * (c + 1)) // KT
                ot = ps_ot.tile([P, QC], dt_f32, tag="ot", name="ps_ot")
                for t in range(nkt):
                    m = t - (QC // KT) * c  # diag block index if >= 0
                    lo_all = KT * m if m >= 0 else 0
                    st = ps_st.tile([P, QC], dt_f32, tag="st", name="ps_att")
                    for n in range(2):
                        lo = max(512 * n, lo_all)
                        hi = 512 * n + 512
                        if lo >= hi:
                            continue
                        nc.tensor.matmul(
                            st[:, lo:hi],
                            kT_sb[fb:fb + HD, S * b2 + KT * t:S * b2 + KT * t + KT],
                            qT_sb[fb:fb + HD, qbase + lo:qbase + hi],
                            start=True, stop=True,
                        )
                    pt = pt_pool.tile([P, QC], dt_bf, tag="pt", name="pt")
                    nc.scalar.activation(
                        out=pt[:, lo_all:QC], in_=st[:, lo_all:QC],
                        func=EXP, scale=0.125,
                    )
                    if m >= 0:
                        nc.vector.tensor_mul(
                            pt[:, KT * m:KT * m + KT],
                            pt[:, KT * m:KT * m + KT],
                            tri_sb[:],
                        )
                    for n in range(2):
                        lo = max(512 * n, lo_all)
                        hi = 512 * n + 512
                        if lo >= hi:
                            continue
                        last_t = (QC // KT) * c + 3 if n == 0 else nkt - 1
                        nc.tensor.matmul(
                            ot[0:HD + 1, lo:hi],
                            v_sb[:, b2, hh, t, 0:HD + 1],
                            pt[:, lo:hi],
                            start=(t == 0), stop=(t == last_t),
                        )
                oun = oun0_sb if hh == 0 else oun1_sb
                nc.vector.tensor_copy(
                    out=oun[:, qbase:qbase + QC],
                    in_=ot[0:HD + 1, :],
                )

            def stage_a2a(b2, c, hh):
                # ship this 1024-token range (2 shards) of one head as soon
                # as that head's attention for (b2, c) is drained
                buf = a2a_in0 if hh == 0 else a2a_in1
                oun = oun0_sb if hh == 0 else oun1_sb
                for j in (2 * (2 * b2 + c), 2 * (2 * b2 + c) + 1):
                    nc.sync.dma_start(
                        buf[j, :, :], oun[:, TOKC * j:TOKC * j + TOKC])

            def collective(idx):
                cin = (a2a_in0, a2a_in1)[idx]
                cout = (a2a_out0, a2a_out1)[idx]
                if single:
                    nc.sync.dma_start(cout[:], cin[:])
                else:
                    nc.gpsimd.collective_compute(
                        "AllToAll",
                        mybir.AluOpType.bypass,
                        replica_groups=[list(range(NCORES))],
                        ins=[cin.opt()],
                        outs=[cout.opt()],
                    )

            # ---- software-pipelined emission: projections for chunk idx+1
            # are spread between the attention groups of chunk idx, so the
            # ACT engine (exp) always has matmul output to chew on while the
            # PE runs projection groups.
            for _rep in range(repeat):
                # hT streamed in per (token-chunk, d-tile) so projections
                # start after the first 2 MB instead of the full 8 MB
                for ch in range(TOK // QC):
                    for a in range(DT):
                        nc.sync.dma_start(
                            hT_sb[:, a, QC * ch:QC * ch + QC],
                            hT[:, a, QC * ch:QC * ch + QC])
                for a in range(DT):
                    nc.sync.dma_start(woT_sb[:, a, :], woT[:, a, :])

                order = [(b2, c) for b2 in range(B) for c in range(S // QC)]
                proj_one(0, "q")
                proj_one(0, "k")
                proj_one(0, "v")
                v_transpose(0, 0)
                for idx, (b2, c) in enumerate(order):
                    nb, nch = order[idx + 1] if idx + 1 < len(order) else (None, None)
                    nxt = idx + 1
                    if nb is not None:
                        proj_one(nxt, "q")
                    attention(b2, 0, c)
                    stage_a2a(b2, c, 0)
                    if nb is not None:
                        proj_one(nxt, "k")
                    if nb is None:
                        # all head-0 shards staged: overlap the first half
                        # of the exchange with the last head-1 attention
                        collective(0)
                    attention(b2, 1, c)
                    stage_a2a(b2, c, 1)
                    if nb is not None:
                        proj_one(nxt, "v")
                        v_transpose(nb, nch)

                collective(1)
                # den16 rows: [8 even heads (2j) | 8 odd heads (2j+1)]
                nc.scalar.dma_start(den16_sb[0:NCORES, :], a2a_out0[:, HD, :])
                nc.scalar.dma_start(
                    den16_sb[NCORES:H, :], a2a_out1[:, HD, :])
                for j in range(NCORES):
                    # split across both HWDGE queues — this unpack is on the
                    # critical path between the collective and W_O
                    nc.sync.dma_start(
                        oT_sb[0:HD, j, :], a2a_out0[j, 0:HD, :])
                    nc.scalar.dma_start(
                        oT_sb[HD:P, j, :], a2a_out1[j, 0:HD, :])

                # ---- normalise: oT *= 1/den (per head, token) --------------
                with nc.allow_low_precision(reason="softmax denom reciprocal to bf16 for PE broadcast"):
                    nc.vector.reciprocal(rec16b_sb[:], den16_sb[:])
                for a in range(DT):
                    bc = ps_st.tile([P, QC], dt_f32, tag="st")
                    nc.tensor.matmul(
                        bc[:, 0:TOKC],
                        sel_sb[:, P * a:P * a + P],
                        rec16b_sb[:],
                        start=True, stop=True,
                    )
                    nc.vector.tensor_mul(
                        oT_sb[:, a, :], oT_sb[:, a, :], bc[:, 0:TOKC])

                # ---- W_O and bias ------------------------------------------
                for a2 in range(DT):
                    ps = ps_st.tile([P, QC], dt_f32, tag="st")
                    for a in range(DT):
                        nc.tensor.matmul(
                            ps[:, 0:TOKC],
                            woT_sb[:, a, P * a2:P * a2 + P],
                            oT_sb[:, a, :],
                            start=(a == 0), stop=(a == DT - 1),
                        )
                    o_out = outp.tile([P, TOKC], dt_f32, tag="oo")
                    if a2 % 2 == 0:
                        nc.vector.tensor_scalar_add(
                            out=o_out[:], in0=ps[:, 0:TOKC],
                            scalar1=bo_sb[:, a2:a2 + 1],
                        )
                    else:
                        nc.scalar.add(
                            out=o_out[:], in_=ps[:, 0:TOKC],
                            add=bo_sb[:, a2:a2 + 1],
                        )
                    eng = nc.sync if a2 % 2 == 0 else nc.scalar
                    eng.dma_start(out_t[P * a2:P * a2 + P, :], o_out[:])

    return nc


def _retile(x):
    """[D, N] -> [P, DT, N] with d = a*128 + p."""
    return np.ascontiguousarray(
        x.reshape(DT, P, -1).transpose(1, 0, 2)
    )


def _prepare_inputs(h, Wq, bq, Wk, bk, Wv, bv, Wo, bo):
    h2 = np.asarray(h, dtype=np.float32).reshape(TOK, D)
    hT_t = _retile(h2.T.astype(BF16))
    woT_t = _retile(np.asarray(Wo, np.float32).T.astype(BF16))
    bo_t = np.ascontiguousarray(
        np.asarray(bo, np.float32).reshape(DT, P).T
    ).astype(np.float32)

    tri = np.triu(np.ones((P, P), np.float32)).astype(BF16)
    # rec16 row r holds head 2r (r<8) or 2(r-8)+1 (r>=8); sel maps it to
    # the partition rows of each d-tile during the PE broadcast
    sel = np.zeros((H, DT * P), np.float32)
    for a in range(DT):
        for p in range(P):
            g = 2 * a + p // HD
            row = g // 2 if g % 2 == 0 else NCORES + g // 2
            sel[row, a * P + p] = 1.0
    sel = sel.astype(BF16)

    in_maps = []
    for c in range(NCORES):
        r = slice(P * c, P * c + P)
        in_maps.append(dict(
            hT=hT_t,
            woT=woT_t,
            wq=_retile(np.asarray(Wq, np.float32)[r].T.astype(BF16)),
            wk=_retile(np.asarray(Wk, np.float32)[r].T.astype(BF16)),
            wv=_retile(np.asarray(Wv, np.float32)[r].T.astype(BF16)),
            bq=np.asarray(bq, np.float32)[r].reshape(P, 1).copy(),
            bk=np.asarray(bk, np.float32)[r].reshape(P, 1).copy(),
            bv=np.asarray(bv, np.float32)[r].reshape(P, 1).copy(),
            bo=bo_t,
            tri=tri,
            sel=sel,
        ))
    return in_maps


def _gather_output(core_outs):
    outT = np.concatenate(core_outs, axis=1)  # [D, TOK]
    return np.ascontiguousarray(outT.T).reshape(B, S, D).astype(np.float32)


LAST_RESULTS = None  # BassKernelResults of the most recent kernel() call


def kernel(h, Wq, bq, Wk, bk, Wv, bv, Wo, bo):
    global LAST_RESULTS
    in_maps = _prepare_inputs(h, Wq, bq, Wk, bk, Wv, bv, Wo, bo)
    nc = _build_nc()
    nc.compile()
    res = run_bass_kernel_spmd(nc, in_maps, core_ids=list(range(NCORES)))
    LAST_RESULTS = res
    return _gather_output([r["out"] for r in res.results])


if __name__ == "__main__":
    d = np.load("/root/problem/inputs_cache.npz")
    out = kernel(**{k: d[k] for k in d.files})
    print("out", out.shape, out.dtype, np.abs(out).max())

